# revision 8
# baseline (speedup 1.0000x reference)
"""MultiHeadGAT Trainium2 kernel (8 NeuronCores, data-parallel over batch).

Reference computation (per batch b of 32, n=512 nodes, d=128 feats, H=8 heads,
HID=64, top-k=16, leaky=0.2):
    h' = (h @ W).reshape(n, H, HID)                      # projection
    ei[g,i] = h'[i,g,:] . a_i[g];  ej[g,j] = h'[j,g,:] . a_j[g]
    e[g,i,j] = leaky_relu(ei[g,i] + ej[g,j])
    mask = topk_16(e, axis=j) | eye(n)
    attn = softmax(where(mask, e, -1e9))
    out = elu(attn @ h')

Key structural facts exploited:
  * leaky_relu is strictly monotone, and e[g,i,:] = leaky(ei[g,i] + ej[g,:]),
    so the top-16 column set J_g is THE SAME for every row i: it is the
    top-16 of the ej[g,:] vector. The attention matrix is therefore
    rank-17-structured: 16 shared columns + the diagonal.
  * softmax rows reduce to 17 candidates; -1e9 fills underflow to exact 0
    in f32, so computing only the 17 candidates is exact.
  * exp(leaky(s)) = max(exp(s), exp(0.2*s))  (monotonicity), so the leaky
    never needs its own pass: two scalar-engine Exp ACTs + one vector max.
  * elu(y) = max(y,0) + exp(min(y,0)) - 1  (exact identity); the two clamps
    are fast tensor_scalar ops that release the PSUM accumulator early.

Fused/pipelined structure: scores for all batches are computed first so the
global top-16 (stage B) overlaps the projection matmuls; stage-C softmax
runs in two 2-batch groups so vector/scalar/gpsimd phases of one group
overlap the other; elementwise work is spread across the three DVE-class
engines; pex/qn run in bf16 for 2x DVE modes.
"""
import sys
import numpy as np

sys.path.insert(0, "/opt/trn_rl_repo")

from contextlib import ExitStack

import concourse.bass as bass
import concourse.tile as tile
from concourse import bacc, mybir
from concourse.bass_utils import run_bass_kernel_spmd

f32 = mybir.dt.float32
bf16 = mybir.dt.bfloat16
AX = mybir.AxisListType
ALU = mybir.AluOpType
AF = mybir.ActivationFunctionType

N_HEADS = 8
HID = 64
TOP_K = 16
SLOPE = 0.2
BS, N, D = 32, 512, 128
CORES = 8
BPC = BS // CORES          # batches per core = 4
NCH = N // 128             # n-chunks = 4
GD = N_HEADS * HID         # 512


def _mid_bcast(ap, insert_at, counts_steps):
    """Insert [step, count] dims into an AP at position insert_at."""
    new = list(ap.ap)
    for step, count in reversed(counts_steps):
        new.insert(insert_at, [step, count])
    return bass.AP(ap.tensor, ap.offset, new)


def build_graph():
    nc = bacc.Bacc("TRN2", target_bir_lowering=False, debug=False)

    hT_ext = nc.dram_tensor("hT", [BPC, D, N], f32, kind="ExternalInput")
    hTb_ext = nc.dram_tensor("hTb", [BPC, D, N], bf16, kind="ExternalInput")
    W_ext = nc.dram_tensor("W", [D, GD], bf16, kind="ExternalInput")
    P_ext = nc.dram_tensor("P", [D, 16], f32, kind="ExternalInput")
    out_ext = nc.dram_tensor("out", [BPC, N, N_HEADS, HID], bf16,
                             kind="ExternalOutput")
    hT = hT_ext.ap()
    hTb = hTb_ext.ap()
    Wap = W_ext.ap()
    Pap = P_ext.ap()
    outap = out_ext.ap()

    with tile.TileContext(nc) as tc, ExitStack() as ctx:
        const = ctx.enter_context(tc.tile_pool(name="const", bufs=1))
        sb = ctx.enter_context(tc.tile_pool(name="sb", bufs=2))
        ps = ctx.enter_context(tc.tile_pool(name="ps", bufs=2, space="PSUM"))

        # ---------------- constants ----------------
        P_sb = const.tile([128, 16], f32)
        nc.sync.dma_start(P_sb[:], Pap)
        W_sb = const.tile([128, GD], bf16)
        nc.sync.dma_start(W_sb[:], Wap)

        # input loads up front; score inputs (ht, f32) first so the top-k
        # chain starts early; htb via the scalar queue to overlap transfers
        ht_sb = []
        htb_sb = []
        for b in range(BPC):
            ht = const.tile([128, N], f32, name=f"ht{b}")
            nc.sync.dma_start(ht[:], hT[b])
            ht_sb.append(ht)
            htb = const.tile([128, N], bf16, name=f"htb{b}")
            nc.scalar.dma_start(htb[:], hTb[b])
            htb_sb.append(htb)

        rowi = const.tile([128, 128], f32)
        nc.gpsimd.iota(rowi[:], [[1, 128]], channel_multiplier=0,
                       allow_small_or_imprecise_dtypes=True)
        coli = const.tile([128, 1], f32)
        nc.gpsimd.iota(coli[:], [[0, 1]], channel_multiplier=1,
                       allow_small_or_imprecise_dtypes=True)
        ident = const.tile([128, 128], f32)
        nc.vector.tensor_scalar(ident[:], rowi[:], coli[:], None,
                                op0=ALU.is_equal)
        identb = const.tile([128, 128], bf16)
        nc.vector.tensor_copy(identb[:], ident[:])
        ones32 = const.tile([32, 128], f32)
        nc.gpsimd.memset(ones32[:], 1.0)

        # block-diag mask: mblk[p, f] = (16*(f//64) <= p <= 16*(f//64)+15)
        colg_lo = const.tile([128, GD], f32)
        nc.gpsimd.iota(colg_lo[:].rearrange("p (g d) -> p g d", g=N_HEADS),
                       [[16, N_HEADS], [0, HID]], channel_multiplier=0,
                       allow_small_or_imprecise_dtypes=True)
        colg_hi = const.tile([128, GD], f32)
        nc.gpsimd.iota(colg_hi[:].rearrange("p (g d) -> p g d", g=N_HEADS),
                       [[16, N_HEADS], [0, HID]], base=15, channel_multiplier=0,
                       allow_small_or_imprecise_dtypes=True)
        mlo = const.tile([128, GD], f32)
        nc.vector.tensor_scalar(mlo[:], colg_lo[:], coli[:], None,
                                op0=ALU.is_le)
        mhi = const.tile([128, GD], f32)
        nc.vector.tensor_scalar(mhi[:], colg_hi[:], coli[:], None,
                                op0=ALU.is_ge)
        mblk = const.tile([128, GD], f32)
        nc.vector.tensor_tensor(mblk[:], mlo[:], mhi[:], op=ALU.mult)

        T = const.tile([32, N], f32)          # ej rows: (b,g) x n
        T2 = const.tile([32, N], f32)
        vals = const.tile([32, 16], f32)

        hp_all = const.tile([128, BPC, NCH, GD], bf16)    # h' per batch
        eij_all = const.tile([128, BPC, NCH, 16], f32)    # [ej(0:8)|ei(8:16)]

        # ---------------- scores first (feeds global top-k) -------------
        # per-node scores recovered by exact PE transpose of the SAME values
        # (bit-identity matters: the one-hot gather compares f32 bits)
        for b in range(BPC):
            eijt_ps = ps.tile([16, N], f32, tag="sm")
            nc.tensor.matmul(eijt_ps[:], P_sb[:], ht_sb[b][:],
                             start=True, stop=True)
            ejt16 = sb.tile([16, N], f32, tag="ejt16")
            nc.scalar.copy(ejt16[:], eijt_ps[:])
            nc.sync.dma_start(T[b * 8:(b + 1) * 8, :], ejt16[0:8, :])

            eij_ps = ps.tile([128, NCH, 16], f32, tag="sm")
            for c in range(NCH):
                nc.tensor.transpose(eij_ps[:, c, :],
                                    ejt16[:, c * 128:(c + 1) * 128],
                                    ident[0:16, 0:16])
            nc.vector.tensor_copy(eij_all[:, b], eij_ps[:])

        # top-16 of ej per (b,g)
        nc.vector.max(vals[:, 0:8], T[:])
        nc.vector.match_replace(T2[:], vals[:, 0:8], T[:], -1e30)
        nc.vector.max(vals[:, 8:16], T2[:])

        # broadcast vals to all 128 partitions: vbc[p, (b,g,c)] = vals[8b+g, c]
        rhsb = const.tile([32, BPC, N_HEADS, 16], f32)
        vals_mid = _mid_bcast(vals[:, 0:16], 1, [[0, BPC], [0, N_HEADS]])
        id_bg = ident[0:32, 0:32].rearrange(
            "p (b g) -> p b g", b=BPC).broadcast_to([32, BPC, N_HEADS, 16])
        nc.vector.tensor_tensor(rhsb[:], vals_mid, id_bg, op=ALU.mult)
        vbc_ps = ps.tile([128, BPC * 128], f32, tag="sm")
        nc.tensor.matmul(vbc_ps[:], ones32[:],
                         rhsb[:].rearrange("k b g c -> k (b g c)"),
                         start=True, stop=True)
        vbc = const.tile([128, BPC, N_HEADS, 16], f32)
        nc.vector.tensor_copy(vbc[:], vbc_ps[:])

        # ---------------- projection ----------------
        for b in range(BPC):
            for p in range(2):                 # chunk pairs (0,1) and (2,3)
                hp_ps = ps.tile([128, 2, GD], f32, tag="big")
                for i in range(2):
                    c = 2 * p + i
                    nc.tensor.matmul(hp_ps[:, i, :],
                                     htb_sb[b][:, c * 128:(c + 1) * 128],
                                     W_sb[:], start=True, stop=True)
                if b % 2 == 0:
                    nc.scalar.copy(hp_all[:, b, 2 * p:2 * p + 2, :], hp_ps[:])
                else:
                    nc.vector.tensor_copy(hp_all[:, b, 2 * p:2 * p + 2, :],
                                          hp_ps[:])

        # ---------------- softmax over 17 candidates (2-batch groups) ---
        GB = 2                                   # batches per group
        pex_t = const.tile([128, BPC, NCH, N_HEADS, 16], bf16)
        pexd_t = const.tile([128, BPC, NCH, N_HEADS], bf16)
        qn = const.tile([128, BPC, NCH, N_HEADS, 16], bf16)
        pdn = const.tile([128, BPC, NCH, N_HEADS], bf16)
        S = const.tile([128, BPC, NCH, N_HEADS, 16], bf16)
        for grp in range(BPC // GB):
            b0 = grp * GB
            bsl = slice(b0, b0 + GB)
            cand16 = sb.tile([128, GB, NCH, N_HEADS, 16], f32, tag="cand16")
            vbc_rep = _mid_bcast(vbc[:, bsl], 2, [[0, NCH]])
            ei_bc = eij_all[:, bsl, :, 8:16].broadcast_to(
                [128, GB, NCH, N_HEADS, 16])
            nc.gpsimd.tensor_tensor(cand16[:], vbc_rep, ei_bc, op=ALU.add)
            candd = sb.tile([128, GB, NCH, N_HEADS], f32, tag="candd")
            nc.vector.tensor_tensor(candd[:], eij_all[:, bsl, :, 0:8],
                                    eij_all[:, bsl, :, 8:16], op=ALU.add)

            # pex = exp(leaky(s)) = max(exp(s), exp(0.2 s))   (bf16)
            pexA = sb.tile([128, GB, NCH, N_HEADS, 16], bf16, tag="pexA")
            nc.scalar.activation(pexA[:], cand16[:], AF.Exp)
            pexB = sb.tile([128, GB, NCH, N_HEADS, 16], bf16, tag="pexB")
            nc.scalar.activation(pexB[:], cand16[:], AF.Exp, scale=SLOPE)
            nc.vector.tensor_tensor(pex_t[:, bsl], pexA[:], pexB[:],
                                    op=ALU.max)
            pexdA = sb.tile([128, GB, NCH, N_HEADS], bf16, tag="pexdA")
            nc.scalar.activation(pexdA[:], candd[:], AF.Exp)
            pexdB = sb.tile([128, GB, NCH, N_HEADS], bf16, tag="pexdB")
            nc.scalar.activation(pexdB[:], candd[:], AF.Exp, scale=SLOPE)
            nc.vector.tensor_tensor(pexd_t[:, bsl], pexdA[:], pexdB[:],
                                    op=ALU.max)

            # diagonal indicator: i not in J_g  <=>  ej_i < t_g
            ind = sb.tile([128, GB, NCH, N_HEADS], f32, tag="ind")
            t_bc = _mid_bcast(
                bass.AP(vbc[:].tensor, vbc[:].offset + b0 * 128 + 15,
                        [vbc[:].ap[0], [128, GB], [16, N_HEADS]]),
                2, [[0, NCH]])
            nc.vector.tensor_tensor(ind[:], eij_all[:, bsl, :, 0:8], t_bc,
                                    op=ALU.is_lt)
            pdiag = sb.tile([128, GB, NCH, N_HEADS], f32, tag="pdiag")
            nc.vector.tensor_tensor(pdiag[:], pexd_t[:, bsl], ind[:],
                                    op=ALU.mult)

            den = sb.tile([128, GB, NCH, N_HEADS], f32, tag="den")
            nc.vector.tensor_reduce(den[:], pex_t[:, bsl], axis=AX.X,
                                    op=ALU.add)
            den2 = sb.tile([128, GB, NCH, N_HEADS], f32, tag="den2")
            nc.vector.tensor_tensor(den2[:], den[:], pdiag[:], op=ALU.add)
            recip = sb.tile([128, GB, NCH, N_HEADS], f32, tag="recip")
            nc.vector.reciprocal(recip[:], den2[:])
            recipb = sb.tile([128, GB, NCH, N_HEADS], bf16, tag="recipb")
            nc.vector.tensor_copy(recipb[:], recip[:])

            nc.vector.tensor_tensor(
                qn[:, bsl], pex_t[:, bsl],
                recipb[:].broadcast_to([128, GB, NCH, N_HEADS, 16]),
                op=ALU.mult)
            nc.vector.tensor_tensor(pdn[:, bsl], pdiag[:], recipb[:],
                                    op=ALU.mult)

            # one-hot S[n, (b,c,g,c16)] = (ej[n,g] == vals[8b+g, c16])
            ej_bc = eij_all[:, bsl, :, 0:8].broadcast_to(
                [128, GB, NCH, N_HEADS, 16])
            nc.vector.tensor_tensor(S[:, bsl], ej_bc, vbc_rep,
                                    op=ALU.is_equal)

        # ---------------- per-batch attention apply ----------------
        for b in range(BPC):
            # gathered rows H_gath[(g,c), :] = h'[j_gc, :], then block-mask
            hg_ps = ps.tile([128, GD], f32, tag="hg")
            for c in range(NCH):
                nc.tensor.matmul(
                    hg_ps[:],
                    S[:, b, c].rearrange("p g c -> p (g c)"),
                    hp_all[:, b, c, :],
                    start=(c == 0), stop=(c == NCH - 1))
            hblk = sb.tile([128, GD], bf16, tag="hblk")
            nc.vector.tensor_tensor(hblk[:], hg_ps[:], mblk[:], op=ALU.mult)

            # qT via PE transpose
            qt_ps = ps.tile([128, NCH, 128], bf16, tag="sm")
            for c in range(NCH):
                nc.tensor.transpose(
                    qt_ps[:, c, :],
                    qn[:, b, c].rearrange("p g c -> p (g c)"), identb[:])
            qt = sb.tile([128, NCH, 128], bf16, tag="qt")
            nc.scalar.copy(qt[:], qt_ps[:])

            # diagonal term dt = h' * pdn (row scale)
            dt = sb.tile([128, NCH, N_HEADS, HID], bf16, tag="dt")
            hp_b = hp_all[:, b].rearrange("p c (g d) -> p c g d", g=N_HEADS)
            pdn_bc = pdn[:, b].broadcast_to([128, NCH, N_HEADS, HID])
            if b % 2 == 0:
                nc.vector.tensor_tensor(dt[:], hp_b, pdn_bc, op=ALU.mult)
            else:
                nc.gpsimd.tensor_tensor(dt[:], hp_b, pdn_bc, op=ALU.mult)

            # attention output + diagonal, then ELU (per half-batch)
            # elu(y) = max(y,0) + exp(min(y,0)) - 1; the two clamps free the
            # PSUM accumulator immediately so the next apply can start
            ot = sb.tile([128, NCH, GD], bf16, tag="ot")
            for h in range(2):
                o_ps = ps.tile([128, 2, GD], f32, tag="big")
                for i in range(2):
                    c = 2 * h + i
                    nc.tensor.matmul(o_ps[:, i, :], qt[:, c, :], hblk[:],
                                     start=True, stop=False)
                    nc.tensor.matmul(
                        o_ps[:, i, :], identb[:],
                        dt[:, c].rearrange("p g d -> p (g d)"),
                        start=False, stop=True)
                r = sb.tile([128, 2, GD], bf16, tag="r")
                m = sb.tile([128, 2, GD], bf16, tag="m")
                vex = sb.tile([128, 2, GD], bf16, tag="vex")
                if (2 * b + h) % 2 == 0:
                    nc.scalar.activation(r[:], o_ps[:], AF.Relu)
                    nc.vector.tensor_scalar_min(m[:], o_ps[:], 0.0)
                    nc.scalar.activation(vex[:], m[:], AF.Exp)
                else:
                    nc.vector.tensor_scalar_max(r[:], o_ps[:], 0.0)
                    nc.scalar.activation(m[:], o_ps[:], AF.Relu, scale=-1.0)
                    nc.scalar.activation(vex[:], m[:], AF.Exp, scale=-1.0)
                nc.vector.scalar_tensor_tensor(
                    ot[:, 2 * h:2 * h + 2, :], vex[:], 1.0, r[:],
                    op0=ALU.subtract, op1=ALU.add)
            nc.sync.dma_start(
                outap[b].rearrange("(c p) g d -> p c g d", c=NCH),
                ot[:].rearrange("p c (g d) -> p c g d", g=N_HEADS))

    nc.compile()
    return nc


_CACHE = {}


def _get_graph():
    if "nc" not in _CACHE:
        _CACHE["nc"] = build_graph()
    return _CACHE["nc"]


def _prep_inputs(h, W, att_a):
    """Host-side marshalling: shard h over cores, transpose to [b,d,n],
    fold attention vectors into P = [W_g @ a_j_g | W_g @ a_i_g]."""
    h = np.asarray(h, dtype=np.float32)
    W = np.asarray(W, dtype=np.float32)
    att_a = np.asarray(att_a, dtype=np.float32)
    P = np.empty((D, 16), dtype=np.float32)
    for g in range(N_HEADS):
        Wg = W[:, g * HID:(g + 1) * HID]
        P[:, g] = Wg @ att_a[g, HID:]      # a_j -> ej (rows 0:8 of EIJT)
        P[:, 8 + g] = Wg @ att_a[g, :HID]  # a_i -> ei
    import ml_dtypes
    Wb = W.astype(ml_dtypes.bfloat16)
    in_maps = []
    for core in range(CORES):
        hs = h[core * BPC:(core + 1) * BPC]            # [4, 512, 128]
        hTs = np.ascontiguousarray(hs.transpose(0, 2, 1))  # [4, 128, 512]
        in_maps.append({"hT": hTs, "hTb": hTs.astype(ml_dtypes.bfloat16),
                        "W": Wb, "P": P})
    return in_maps


def kernel(h, W, att_a):
    nc = _get_graph()
    in_maps = _prep_inputs(h, W, att_a)
    res = run_bass_kernel_spmd(nc, in_maps, list(range(CORES))).results
    outs = [r["out"].transpose(0, 2, 1, 3) for r in res]  # [4,H,n,d] each
    return np.ascontiguousarray(np.concatenate(outs, axis=0))


# revision 9
# speedup vs baseline: 1.0069x; 1.0069x over previous
"""MultiHeadGAT Trainium2 kernel (8 NeuronCores, data-parallel over batch).

Reference computation (per batch b of 32, n=512 nodes, d=128 feats, H=8 heads,
HID=64, top-k=16, leaky=0.2):
    h' = (h @ W).reshape(n, H, HID)                      # projection
    ei[g,i] = h'[i,g,:] . a_i[g];  ej[g,j] = h'[j,g,:] . a_j[g]
    e[g,i,j] = leaky_relu(ei[g,i] + ej[g,j])
    mask = topk_16(e, axis=j) | eye(n)
    attn = softmax(where(mask, e, -1e9))
    out = elu(attn @ h')

Key structural facts exploited:
  * leaky_relu is strictly monotone, and e[g,i,:] = leaky(ei[g,i] + ej[g,:]),
    so the top-16 column set J_g is THE SAME for every row i: it is the
    top-16 of the ej[g,:] vector. The attention matrix is therefore
    rank-17-structured: 16 shared columns + the diagonal.
  * softmax rows reduce to 17 candidates; -1e9 fills underflow to exact 0
    in f32, so computing only the 17 candidates is exact.
  * exp(leaky(s)) = max(exp(s), exp(0.2*s))  (monotonicity), so the leaky
    never needs its own pass: two scalar-engine Exp ACTs + one vector max.
  * elu(y) = max(y,0) + exp(min(y,0)) - 1  (exact identity); the two clamps
    are fast tensor_scalar ops that release the PSUM accumulator early.

Fused/pipelined structure: scores for all batches are computed first so the
global top-16 (stage B) overlaps the projection matmuls; stage-C softmax
runs in two 2-batch groups so vector/scalar/gpsimd phases of one group
overlap the other; elementwise work is spread across the three DVE-class
engines; pex/qn run in bf16 for 2x DVE modes.
"""
import sys
import numpy as np

sys.path.insert(0, "/opt/trn_rl_repo")

from contextlib import ExitStack

import concourse.bass as bass
import concourse.tile as tile
from concourse import bacc, mybir
from concourse.bass_utils import run_bass_kernel_spmd

f32 = mybir.dt.float32
bf16 = mybir.dt.bfloat16
AX = mybir.AxisListType
ALU = mybir.AluOpType
AF = mybir.ActivationFunctionType

N_HEADS = 8
HID = 64
TOP_K = 16
SLOPE = 0.2
BS, N, D = 32, 512, 128
CORES = 8
BPC = BS // CORES          # batches per core = 4
NCH = N // 128             # n-chunks = 4
GD = N_HEADS * HID         # 512


def _mid_bcast(ap, insert_at, counts_steps):
    """Insert [step, count] dims into an AP at position insert_at."""
    new = list(ap.ap)
    for step, count in reversed(counts_steps):
        new.insert(insert_at, [step, count])
    return bass.AP(ap.tensor, ap.offset, new)


def _apply_batch(nc, ps, sb, outap, qt, hblk, dt, identb, b):
    ot = sb.tile([128, NCH, GD], bf16, tag="ot")
    for h in range(2):
        o_ps = ps.tile([128, 2, GD], f32, tag="big")
        for i in range(2):
            c = 2 * h + i
            nc.tensor.matmul(o_ps[:, i, :], qt[:, c, :], hblk[:],
                             start=True, stop=False)
            nc.tensor.matmul(
                o_ps[:, i, :], identb[:],
                dt[:, c].rearrange("p g d -> p (g d)"),
                start=False, stop=True)
        # elu(y) = max(y, exp(min(y,0)) - 1);  min(y,0) = -relu(-y)
        rneg = sb.tile([128, 2, GD], bf16, tag="m")
        nc.scalar.activation(rneg[:], o_ps[:], AF.Relu, scale=-1.0)
        vex = sb.tile([128, 2, GD], bf16, tag="vex")
        nc.scalar.activation(vex[:], rneg[:], AF.Exp, scale=-1.0)
        nc.vector.scalar_tensor_tensor(
            ot[:, 2 * h:2 * h + 2, :], vex[:], 1.0, o_ps[:],
            op0=ALU.subtract, op1=ALU.max)
    nc.sync.dma_start(
        outap[b].rearrange("(c p) g d -> p c g d", c=NCH),
        ot[:].rearrange("p c (g d) -> p c g d", g=N_HEADS))


def build_graph():
    nc = bacc.Bacc("TRN2", target_bir_lowering=False, debug=False)

    hT_ext = nc.dram_tensor("hT", [BPC, D, N], f32, kind="ExternalInput")
    hTb_ext = nc.dram_tensor("hTb", [BPC, D, N], bf16, kind="ExternalInput")
    W_ext = nc.dram_tensor("W", [D, GD], bf16, kind="ExternalInput")
    P_ext = nc.dram_tensor("P", [D, 16], f32, kind="ExternalInput")
    out_ext = nc.dram_tensor("out", [BPC, N, N_HEADS, HID], bf16,
                             kind="ExternalOutput")
    hT = hT_ext.ap()
    hTb = hTb_ext.ap()
    Wap = W_ext.ap()
    Pap = P_ext.ap()
    outap = out_ext.ap()

    with tile.TileContext(nc) as tc, ExitStack() as ctx:
        const = ctx.enter_context(tc.tile_pool(name="const", bufs=1))
        sb = ctx.enter_context(tc.tile_pool(name="sb", bufs=2))
        ps = ctx.enter_context(tc.tile_pool(name="ps", bufs=2, space="PSUM"))

        # ---------------- constants ----------------
        P_sb = const.tile([128, 16], f32)
        nc.sync.dma_start(P_sb[:], Pap)
        W_sb = const.tile([128, GD], bf16)
        nc.sync.dma_start(W_sb[:], Wap)

        # input loads up front; score inputs (ht, f32) first so the top-k
        # chain starts early; htb via the scalar queue to overlap transfers
        ht_sb = []
        htb_sb = []
        for b in range(BPC):
            ht = const.tile([128, N], f32, name=f"ht{b}")
            nc.sync.dma_start(ht[:], hT[b])
            ht_sb.append(ht)
            htb = const.tile([128, N], bf16, name=f"htb{b}")
            nc.scalar.dma_start(htb[:], hTb[b])
            htb_sb.append(htb)

        rowi = const.tile([128, 128], f32)
        nc.gpsimd.iota(rowi[:], [[1, 128]], channel_multiplier=0,
                       allow_small_or_imprecise_dtypes=True)
        coli = const.tile([128, 1], f32)
        nc.gpsimd.iota(coli[:], [[0, 1]], channel_multiplier=1,
                       allow_small_or_imprecise_dtypes=True)
        ident = const.tile([128, 128], f32)
        nc.vector.tensor_scalar(ident[:], rowi[:], coli[:], None,
                                op0=ALU.is_equal)
        identb = const.tile([128, 128], bf16)
        nc.vector.tensor_copy(identb[:], ident[:])
        ones32 = const.tile([32, 128], f32)
        nc.gpsimd.memset(ones32[:], 1.0)

        # block-diag mask: mblk[p, f] = (16*(f//64) <= p <= 16*(f//64)+15)
        colg_lo = const.tile([128, GD], f32)
        nc.gpsimd.iota(colg_lo[:].rearrange("p (g d) -> p g d", g=N_HEADS),
                       [[16, N_HEADS], [0, HID]], channel_multiplier=0,
                       allow_small_or_imprecise_dtypes=True)
        colg_hi = const.tile([128, GD], f32)
        nc.gpsimd.iota(colg_hi[:].rearrange("p (g d) -> p g d", g=N_HEADS),
                       [[16, N_HEADS], [0, HID]], base=15, channel_multiplier=0,
                       allow_small_or_imprecise_dtypes=True)
        mlo = const.tile([128, GD], f32)
        nc.vector.tensor_scalar(mlo[:], colg_lo[:], coli[:], None,
                                op0=ALU.is_le)
        mhi = const.tile([128, GD], f32)
        nc.vector.tensor_scalar(mhi[:], colg_hi[:], coli[:], None,
                                op0=ALU.is_ge)
        mblk = const.tile([128, GD], f32)
        nc.vector.tensor_tensor(mblk[:], mlo[:], mhi[:], op=ALU.mult)

        T = const.tile([32, N], f32)          # ej rows: (b,g) x n
        T2 = const.tile([32, N], f32)
        vals = const.tile([32, 16], f32)

        hp_all = const.tile([128, BPC, NCH, GD], bf16)    # h' per batch
        eij_all = const.tile([128, BPC, NCH, 16], f32)    # [ej(0:8)|ei(8:16)]

        # ---------------- scores first (feeds global top-k) -------------
        # per-node scores recovered by exact PE transpose of the SAME values
        # (bit-identity matters: the one-hot gather compares f32 bits)
        for b in range(BPC):
            eijt_ps = ps.tile([16, N], f32, tag="sm")
            nc.tensor.matmul(eijt_ps[:], P_sb[:], ht_sb[b][:],
                             start=True, stop=True)
            ejt16 = sb.tile([16, N], f32, tag="ejt16")
            nc.scalar.copy(ejt16[:], eijt_ps[:])
            nc.sync.dma_start(T[b * 8:(b + 1) * 8, :], ejt16[0:8, :])

            eij_ps = ps.tile([128, NCH, 16], f32, tag="sm")
            for c in range(NCH):
                nc.tensor.transpose(eij_ps[:, c, :],
                                    ejt16[:, c * 128:(c + 1) * 128],
                                    ident[0:16, 0:16])
            nc.vector.tensor_copy(eij_all[:, b], eij_ps[:])

        # top-16 of ej per (b,g)
        nc.vector.max(vals[:, 0:8], T[:])
        nc.vector.match_replace(T2[:], vals[:, 0:8], T[:], -1e30)
        nc.vector.max(vals[:, 8:16], T2[:])

        # broadcast vals to all 128 partitions: vbc[p, (b,g,c)] = vals[8b+g, c]
        rhsb = const.tile([32, BPC, N_HEADS, 16], f32)
        vals_mid = _mid_bcast(vals[:, 0:16], 1, [[0, BPC], [0, N_HEADS]])
        id_bg = ident[0:32, 0:32].rearrange(
            "p (b g) -> p b g", b=BPC).broadcast_to([32, BPC, N_HEADS, 16])
        nc.vector.tensor_tensor(rhsb[:], vals_mid, id_bg, op=ALU.mult)
        vbc_ps = ps.tile([128, BPC * 128], f32, tag="sm")
        nc.tensor.matmul(vbc_ps[:], ones32[:],
                         rhsb[:].rearrange("k b g c -> k (b g c)"),
                         start=True, stop=True)
        vbc = const.tile([128, BPC, N_HEADS, 16], f32)
        nc.vector.tensor_copy(vbc[:], vbc_ps[:])

        # ---------------- projection ----------------
        for b in range(BPC):
            for p in range(2):                 # chunk pairs (0,1) and (2,3)
                hp_ps = ps.tile([128, 2, GD], f32, tag="big")
                for i in range(2):
                    c = 2 * p + i
                    nc.tensor.matmul(hp_ps[:, i, :],
                                     htb_sb[b][:, c * 128:(c + 1) * 128],
                                     W_sb[:], start=True, stop=True)
                if b % 2 == 0:
                    nc.scalar.copy(hp_all[:, b, 2 * p:2 * p + 2, :], hp_ps[:])
                else:
                    nc.vector.tensor_copy(hp_all[:, b, 2 * p:2 * p + 2, :],
                                          hp_ps[:])

        # ---------------- softmax over 17 candidates (2-batch groups) ---
        GB = 2                                   # batches per group
        pex_t = const.tile([128, BPC, NCH, N_HEADS, 16], bf16)
        pexd_t = const.tile([128, BPC, NCH, N_HEADS], bf16)
        qn = const.tile([128, BPC, NCH, N_HEADS, 16], bf16)
        pdn = const.tile([128, BPC, NCH, N_HEADS], bf16)
        S = const.tile([128, BPC, NCH, N_HEADS, 16], bf16)
        for grp in range(BPC // GB):
            b0 = grp * GB
            bsl = slice(b0, b0 + GB)
            cand16 = sb.tile([128, GB, NCH, N_HEADS, 16], f32, tag="cand16")
            vbc_rep = _mid_bcast(vbc[:, bsl], 2, [[0, NCH]])
            ei_bc = eij_all[:, bsl, :, 8:16].broadcast_to(
                [128, GB, NCH, N_HEADS, 16])
            nc.gpsimd.tensor_tensor(cand16[:], vbc_rep, ei_bc, op=ALU.add)
            candd = sb.tile([128, GB, NCH, N_HEADS], f32, tag="candd")
            nc.vector.tensor_tensor(candd[:], eij_all[:, bsl, :, 0:8],
                                    eij_all[:, bsl, :, 8:16], op=ALU.add)

            # pex = exp(leaky(s)) = max(exp(s), exp(0.2 s))   (bf16)
            pexA = sb.tile([128, GB, NCH, N_HEADS, 16], bf16, tag="pexA")
            nc.scalar.activation(pexA[:], cand16[:], AF.Exp)
            pexB = sb.tile([128, GB, NCH, N_HEADS, 16], bf16, tag="pexB")
            nc.scalar.activation(pexB[:], cand16[:], AF.Exp, scale=SLOPE)
            nc.vector.tensor_tensor(pex_t[:, bsl], pexA[:], pexB[:],
                                    op=ALU.max)
            pexdA = sb.tile([128, GB, NCH, N_HEADS], bf16, tag="pexdA")
            nc.scalar.activation(pexdA[:], candd[:], AF.Exp)
            pexdB = sb.tile([128, GB, NCH, N_HEADS], bf16, tag="pexdB")
            nc.scalar.activation(pexdB[:], candd[:], AF.Exp, scale=SLOPE)
            nc.vector.tensor_tensor(pexd_t[:, bsl], pexdA[:], pexdB[:],
                                    op=ALU.max)

            # diagonal indicator: i not in J_g  <=>  ej_i < t_g
            ind = sb.tile([128, GB, NCH, N_HEADS], f32, tag="ind")
            t_bc = _mid_bcast(
                bass.AP(vbc[:].tensor, vbc[:].offset + b0 * 128 + 15,
                        [vbc[:].ap[0], [128, GB], [16, N_HEADS]]),
                2, [[0, NCH]])
            nc.vector.tensor_tensor(ind[:], eij_all[:, bsl, :, 0:8], t_bc,
                                    op=ALU.is_lt)
            pdiag = sb.tile([128, GB, NCH, N_HEADS], f32, tag="pdiag")
            nc.vector.tensor_tensor(pdiag[:], pexd_t[:, bsl], ind[:],
                                    op=ALU.mult)

            den = sb.tile([128, GB, NCH, N_HEADS], f32, tag="den")
            nc.vector.tensor_reduce(den[:], pex_t[:, bsl], axis=AX.X,
                                    op=ALU.add)
            den2 = sb.tile([128, GB, NCH, N_HEADS], f32, tag="den2")
            nc.vector.tensor_tensor(den2[:], den[:], pdiag[:], op=ALU.add)
            recip = sb.tile([128, GB, NCH, N_HEADS], f32, tag="recip")
            nc.vector.reciprocal(recip[:], den2[:])
            recipb = sb.tile([128, GB, NCH, N_HEADS], bf16, tag="recipb")
            nc.vector.tensor_copy(recipb[:], recip[:])

            nc.vector.tensor_tensor(
                qn[:, bsl], pex_t[:, bsl],
                recipb[:].broadcast_to([128, GB, NCH, N_HEADS, 16]),
                op=ALU.mult)
            nc.vector.tensor_tensor(pdn[:, bsl], pdiag[:], recipb[:],
                                    op=ALU.mult)

            # one-hot S[n, (b,c,g,c16)] = (ej[n,g] == vals[8b+g, c16])
            ej_bc = eij_all[:, bsl, :, 0:8].broadcast_to(
                [128, GB, NCH, N_HEADS, 16])
            nc.vector.tensor_tensor(S[:, bsl], ej_bc, vbc_rep,
                                    op=ALU.is_equal)

        # ---------------- per-batch attention apply ----------------
        # PE program order interleaves gather/transpose groups of later
        # batches between apply groups of earlier ones so the PE queue
        # stays dense (HAM stays un-throttled) while vector/scalar run the
        # per-batch epilogues.
        hblk_t = []
        qt_t = []
        dt_t = []
        for b in range(BPC):
            hg_ps = ps.tile([128, GD], f32, tag="hg")
            for c in range(NCH):
                nc.tensor.matmul(
                    hg_ps[:],
                    S[:, b, c].rearrange("p g c -> p (g c)"),
                    hp_all[:, b, c, :],
                    start=(c == 0), stop=(c == NCH - 1))
            hblk = sb.tile([128, GD], bf16, tag="hblk")
            nc.vector.tensor_tensor(hblk[:], hg_ps[:], mblk[:], op=ALU.mult)
            hblk_t.append(hblk)

            qt_ps = ps.tile([128, NCH, 128], bf16, tag="sm")
            for c in range(NCH):
                nc.tensor.transpose(
                    qt_ps[:, c, :],
                    qn[:, b, c].rearrange("p g c -> p (g c)"), identb[:])
            qt = sb.tile([128, NCH, 128], bf16, tag="qt")
            nc.vector.tensor_copy(qt[:], qt_ps[:])
            qt_t.append(qt)

            dt = sb.tile([128, NCH, N_HEADS, HID], bf16, tag="dt")
            hp_b = hp_all[:, b].rearrange("p c (g d) -> p c g d", g=N_HEADS)
            pdn_bc = pdn[:, b].broadcast_to([128, NCH, N_HEADS, HID])
            nc.gpsimd.tensor_tensor(dt[:], hp_b, pdn_bc, op=ALU.mult)
            dt_t.append(dt)

            if b >= 1:
                _apply_batch(nc, ps, sb, outap, qt_t[b - 1], hblk_t[b - 1],
                             dt_t[b - 1], identb, b - 1)
        _apply_batch(nc, ps, sb, outap, qt_t[BPC - 1], hblk_t[BPC - 1],
                     dt_t[BPC - 1], identb, BPC - 1)

    nc.compile()
    return nc


_CACHE = {}


def _get_graph():
    if "nc" not in _CACHE:
        _CACHE["nc"] = build_graph()
    return _CACHE["nc"]


def _prep_inputs(h, W, att_a):
    """Host-side marshalling: shard h over cores, transpose to [b,d,n],
    fold attention vectors into P = [W_g @ a_j_g | W_g @ a_i_g]."""
    h = np.asarray(h, dtype=np.float32)
    W = np.asarray(W, dtype=np.float32)
    att_a = np.asarray(att_a, dtype=np.float32)
    P = np.empty((D, 16), dtype=np.float32)
    for g in range(N_HEADS):
        Wg = W[:, g * HID:(g + 1) * HID]
        P[:, g] = Wg @ att_a[g, HID:]      # a_j -> ej (rows 0:8 of EIJT)
        P[:, 8 + g] = Wg @ att_a[g, :HID]  # a_i -> ei
    import ml_dtypes
    Wb = W.astype(ml_dtypes.bfloat16)
    in_maps = []
    for core in range(CORES):
        hs = h[core * BPC:(core + 1) * BPC]            # [4, 512, 128]
        hTs = np.ascontiguousarray(hs.transpose(0, 2, 1))  # [4, 128, 512]
        in_maps.append({"hT": hTs, "hTb": hTs.astype(ml_dtypes.bfloat16),
                        "W": Wb, "P": P})
    return in_maps


def kernel(h, W, att_a):
    nc = _get_graph()
    in_maps = _prep_inputs(h, W, att_a)
    res = run_bass_kernel_spmd(nc, in_maps, list(range(CORES))).results
    outs = [r["out"].transpose(0, 2, 1, 3) for r in res]  # [4,H,n,d] each
    return np.ascontiguousarray(np.concatenate(outs, axis=0))
